# revision 3
# baseline (speedup 1.0000x reference)
"""Trainium2 Bass kernel for nn_Alignment_vector (cross-attention alignment).

Data-parallel over batch across 8 NeuronCores (4 batches each). All three
matmuls run in bf16 on the TensorEngine; elementwise/normalization work is
fp32 on DVE/ACT. Transposed operands (the attention contraction is over the
contiguous `d` axis) are produced by staging bf16 copies to DRAM and loading
them back through the hardware x-bar transpose DMA.

Math note: the softmax denominator cancels inside the following
l2_normalize, so softmax is computed as a bare exp().
"""

import numpy as np

import concourse.bacc as bacc
import concourse.tile as tile
import concourse.mybir as mybir
from concourse.bass_utils import run_bass_kernel_spmd

f32 = mybir.dt.float32
bf16 = mybir.dt.bfloat16
AF = mybir.ActivationFunctionType
ALU = mybir.AluOpType

B, NCORES = 32, 8
BPC = B // NCORES            # batches per core
LQ, LS, D, S = 512, 1024, 1024, 256
NQ, NS, ND = LQ // 128, LS // 128, D // 128   # 4, 8, 8
EPS = 1e-8

LAST_EXEC_TIME_NS = None


def _build(smooth: float):
    nc = bacc.Bacc("TRN2", target_bir_lowering=False, debug=False)

    q_d = nc.dram_tensor("query", (BPC, LQ, D), f32, kind="ExternalInput").ap()
    c_d = nc.dram_tensor("context", (BPC, LS, D), f32, kind="ExternalInput").ap()
    m_d = nc.dram_tensor("matrix", (BPC, LQ, D), f32, kind="ExternalInput").ap()
    W_d = nc.dram_tensor("W", (S, D), f32, kind="ExternalInput").ap()
    bias_d = nc.dram_tensor("b", (S,), f32, kind="ExternalInput").ap()
    out_d = nc.dram_tensor("out", (BPC, LQ, S), f32, kind="ExternalOutput").ap()

    # bf16 staging areas for x-bar transposes
    qm_s = nc.dram_tensor("qm_s", (BPC, LQ, D), bf16, kind="Internal").ap()
    cx_s = nc.dram_tensor("cx_s", (BPC, LS, D), bf16, kind="Internal").ap()
    sim_s = nc.dram_tensor("sim_s", (BPC, LQ, D), bf16, kind="Internal").ap()
    W_s = nc.dram_tensor("W_s", (S, D), bf16, kind="Internal").ap()

    with tile.TileContext(nc) as tc:
        from contextlib import ExitStack
        with ExitStack() as ctx:
            p = lambda *a, **k: ctx.enter_context(tc.tile_pool(*a, **k))
            qf_pool = p(name="qf", bufs=2)
            mf_pool = p(name="mf", bufs=1)
            qm_pool = p(name="qm", bufs=1)
            cx_pool = p(name="cx", bufs=2)
            cT_pool = p(name="cT", bufs=2)
            qT_pool = p(name="qT", bufs=2)
            al_pool = p(name="al", bufs=1)
            ee_pool = p(name="ee", bufs=2)
            sim_pool = p(name="sim", bufs=1)
            simT_pool = p(name="simT", bufs=1)
            wrk_pool = p(name="wrk", bufs=2)
            sm_pool = p(name="sm", bufs=2)
            out_pool = p(name="outp", bufs=2)
            const_pool = p(name="const", bufs=1)
            psA_pool = p(name="psA", bufs=2, space="PSUM")
            psW_pool = p(name="psW", bufs=2, space="PSUM")
            psO_pool = p(name="psO", bufs=2, space="PSUM")
            # ---- once-per-core constants ----
            # W -> bf16 -> DRAM -> x-bar transpose -> WT[d, S] tiles
            Wsb = qm_pool.tile([128, S // 128, D], bf16, tag="qm")
            nc.gpsimd.dma_start(
                Wsb[:], W_d.rearrange("(t p) d -> p t d", p=128))
            nc.gpsimd.dma_start(
                W_s.rearrange("(t p) d -> p t d", p=128), Wsb[:])
            WT = const_pool.tile([128, ND, S], bf16)
            nc.sync.dma_start(WT[:], W_s, transpose=True)

            # bias broadcast to [128, S] via K=1 matmul with a ones column
            ones_c = const_pool.tile([1, 128], f32)
            nc.vector.memset(ones_c[:], 1.0)
            b_sb = const_pool.tile([1, S], f32)
            nc.gpsimd.dma_start(b_sb[:], bias_d.rearrange("(o s) -> o s", o=1))
            ps_b = psO_pool.tile([128, S], f32, tag="psO")
            nc.tensor.matmul(ps_b[:], lhsT=ones_c[:], rhs=b_sb[:],
                             start=True, stop=True)
            bB = const_pool.tile([128, S], f32)
            nc.vector.tensor_copy(bB[:], ps_b[:])

            for bi in range(BPC):
                # ---- stage A: loads ----
                qf = qf_pool.tile([128, NQ, D], f32)
                nc.gpsimd.dma_start(
                    qf[:], q_d[bi].rearrange("(t p) d -> p t d", p=128))
                mf = mf_pool.tile([128, NQ, D], bf16)
                nc.gpsimd.dma_start(
                    mf[:], m_d[bi].rearrange("(t p) d -> p t d", p=128))
                cx = cx_pool.tile([128, NS, D], bf16)
                nc.gpsimd.dma_start(
                    cx[:], c_d[bi].rearrange("(t p) d -> p t d", p=128))

                # ---- stage B: qm product, staging stores, transposes ----
                qm = qm_pool.tile([128, NQ, D], bf16, tag="qm")
                nc.vector.tensor_tensor(out=qm[:], in0=qf[:], in1=mf[:],
                                        op=ALU.mult)
                nc.gpsimd.dma_start(
                    qm_s[bi].rearrange("(t p) d -> p t d", p=128), qm[:])
                nc.gpsimd.dma_start(
                    cx_s[bi].rearrange("(t p) d -> p t d", p=128), cx[:])
                qT = qT_pool.tile([128, ND, LQ], bf16)
                nc.sync.dma_start(qT[:], qm_s[bi], transpose=True)
                cT = cT_pool.tile([128, ND, LS], bf16)
                nc.sync.dma_start(cT[:], cx_s[bi], transpose=True)

                # ---- stage C: attn = lrelu(context @ qm.T), row-normalize,
                #      E = exp(smooth * attn / ||row||) ----
                AL = al_pool.tile([128, NS, LQ], bf16)
                ss = sm_pool.tile([128, NS], f32, tag="ss")
                for m in range(NS):
                    psA = psA_pool.tile([128, LQ], f32)
                    for k in range(ND):
                        nc.tensor.matmul(
                            psA[:], lhsT=cT[:, k, 128 * m:128 * (m + 1)],
                            rhs=qT[:, k, :],
                            start=(k == 0), stop=(k == ND - 1))
                    t01 = wrk_pool.tile([128, LQ], bf16, tag="t01")
                    nc.scalar.mul(t01[:], psA[:], 0.1)
                    nc.vector.tensor_tensor(out=AL[:, m, :], in0=psA[:],
                                            in1=t01[:], op=ALU.max)
                    sqd = wrk_pool.tile([128, LQ], bf16, tag="sqd")
                    nc.scalar.activation(sqd[:], AL[:, m, :], AF.Square,
                                         accum_out=ss[:, m:m + 1])
                rs = sm_pool.tile([128, NS], f32, tag="rs")
                nc.scalar.sqrt(rs[:], ss[:])
                nc.vector.reciprocal(rs[:], rs[:])
                nc.vector.tensor_scalar_mul(rs[:], rs[:], float(smooth))
                E = ee_pool.tile([128, NS, LQ], bf16)
                for m in range(NS):
                    nc.scalar.activation(E[:, m, :], AL[:, m, :], AF.Exp,
                                         scale=rs[:, m:m + 1])

                # ---- stage D: wc = E.T @ context, l2-normalize rows,
                #      sim = (query - wcn)^2 ----
                sim = sim_pool.tile([128, NQ, D], bf16)
                for mq in range(NQ):
                    psW = psW_pool.tile([128, D], f32)
                    for n in range(2):
                        sl = slice(512 * n, 512 * (n + 1))
                        for k in range(NS):
                            nc.tensor.matmul(
                                psW[:, sl],
                                lhsT=E[:, k, 128 * mq:128 * (mq + 1)],
                                rhs=cx[:, k, sl],
                                start=(k == 0), stop=(k == NS - 1))
                    ssw = sm_pool.tile([128, 2], f32, tag="ssw")
                    sq0 = wrk_pool.tile([128, 512], bf16, tag="sqd")
                    nc.scalar.activation(sq0[:], psW[:, 0:512], AF.Square,
                                         accum_out=ssw[:, 0:1])
                    sq1 = wrk_pool.tile([128, 512], bf16, tag="sqd")
                    nc.scalar.activation(sq1[:], psW[:, 512:1024], AF.Square,
                                         accum_out=ssw[:, 1:2])
                    g = sm_pool.tile([128, 1], f32, tag="g")
                    nc.vector.tensor_tensor(out=g[:], in0=ssw[:, 0:1],
                                            in1=ssw[:, 1:2], op=ALU.add)
                    nc.scalar.sqrt(g[:], g[:])
                    nc.vector.reciprocal(g[:], g[:])
                    wn = wrk_pool.tile([128, D], bf16, tag="wn")
                    nc.vector.tensor_scalar_mul(wn[:], psW[:], g[:])
                    for n in range(2):
                        sl = slice(512 * n, 512 * (n + 1))
                        tt = wrk_pool.tile([128, 512], f32, tag="tt")
                        nc.vector.tensor_tensor(out=tt[:], in0=qf[:, mq, sl],
                                                in1=wn[:, sl], op=ALU.subtract)
                        nc.scalar.activation(sim[:, mq, sl], tt[:], AF.Square)
                nc.gpsimd.dma_start(
                    sim_s[bi].rearrange("(t p) d -> p t d", p=128), sim[:])
                simT = simT_pool.tile([128, ND, LQ], bf16)
                nc.sync.dma_start(simT[:], sim_s[bi], transpose=True)

                # ---- stage E: out = l2norm(sim @ W.T + b) ----
                outT = out_pool.tile([128, NQ, S], f32)
                for mq in range(NQ):
                    psO = psO_pool.tile([128, S], f32, tag="psO")
                    for k in range(ND):
                        nc.tensor.matmul(
                            psO[:], lhsT=simT[:, k, 128 * mq:128 * (mq + 1)],
                            rhs=WT[:, k, :],
                            start=(k == 0), stop=(k == ND - 1))
                    t2 = wrk_pool.tile([128, S], f32, tag="t2")
                    nc.vector.tensor_tensor(out=t2[:], in0=psO[:], in1=bB[:],
                                            op=ALU.add)
                    sq3 = wrk_pool.tile([128, S], bf16, tag="sqd")
                    ss3 = sm_pool.tile([128, 1], f32, tag="ss3")
                    nc.scalar.activation(sq3[:], t2[:], AF.Square,
                                         accum_out=ss3[:])
                    nc.scalar.sqrt(ss3[:], ss3[:])
                    nc.vector.tensor_scalar_add(ss3[:], ss3[:], EPS)
                    nc.vector.reciprocal(ss3[:], ss3[:])
                    nc.vector.tensor_scalar_mul(outT[:, mq, :], t2[:], ss3[:])
                nc.gpsimd.dma_start(
                    out_d[bi].rearrange("(t p) s -> p t s", p=128), outT[:])

    nc.compile()
    return nc


_NC_CACHE: dict = {}


def kernel(query, context, matrix, W, b, smooth):
    global LAST_EXEC_TIME_NS
    sm = float(smooth)
    nc = _NC_CACHE.get(sm)
    if nc is None:
        nc = _build(sm)
        _NC_CACHE[sm] = nc

    query = np.ascontiguousarray(query, dtype=np.float32)
    context = np.ascontiguousarray(context, dtype=np.float32)
    matrix = np.ascontiguousarray(matrix, dtype=np.float32)
    W = np.ascontiguousarray(W, dtype=np.float32)
    b = np.ascontiguousarray(b, dtype=np.float32)

    in_maps = []
    for c in range(NCORES):
        sl = slice(c * BPC, (c + 1) * BPC)
        in_maps.append({
            "query": query[sl],
            "context": context[sl],
            "matrix": matrix[sl],
            "W": W,
            "b": b,
        })
    res = run_bass_kernel_spmd(nc, in_maps, core_ids=list(range(NCORES)))
    LAST_EXEC_TIME_NS = res.exec_time_ns
    out = np.concatenate([r["out"] for r in res.results], axis=0)
    return out
